# revision 1
# baseline (speedup 1.0000x reference)
"""Trainium2 Bass kernel for nn_BaseRGCNHetero (3-layer heterogeneous RGCN).

Strategy (8 NeuronCores, SPMD):
  - Destination-shard the nodes: core c owns rows [c*N/8, (c+1)*N/8) of every
    node type; all edges whose dst is in the shard are processed there, so
    per-relation aggregates need no cross-core reduction.
  - Aggregate-first algebra: agg[dst] = (sum_{e->dst} h[src]) @ W_r * inv_deg,
    sharing one bf16 gather table per source ntype (drug, gene) per layer.
  - After each layer the drug/gene h-shards are AllGathered (bf16) into
    per-core DRAM gather tables for the next layer.
  - Segment sums: host lays edges out as a padded, degree-bucketed slot
    stream per (relation, 128-dst window, src bank, 32-dst subgroup).
    dma_gather (bf16, transpose=True) produces feature-major tiles; VectorE
    tensor_reduce over the innermost (slot) axis yields segment sums.  Pad
    slots point at an all-zero table row.
  - Per (relation, window): a one-hot "unpermute * inv_deg" matrix is built
    by a fused tensor_scalar(is_equal, mult); two matmuls apply W_r and the
    window permutation back to natural dst order, accumulating into a
    feature-major fp32 SBUF accumulator; the self-loop h @ L is one more
    matmul; bias+relu is a fused ScalarE activation per window.
"""
import sys
import types
import numpy as np
import ml_dtypes
from contextlib import ExitStack

import concourse.bass as bass
import concourse.bacc as bacc
import concourse.tile as tile
from concourse import mybir, library_config

BF16 = ml_dtypes.bfloat16
P = 128
SUBG = 16          # dsts per reduce subgroup
NSUB = P // SUBG   # subgroups per window
GCAP = 8192        # target max slots per dma_gather

CFG = dict(
    N={"drug": 20000, "gene": 50000, "disease": 10000},
    MOD={"drug": 1024, "gene": 768, "disease": 512},
    D_IN=128, D_H=128, D_OUT=64,
    RELS=[("drug", "disease", "dd"), ("drug", "drug", "ddr"),
          ("drug", "gene", "dg"), ("gene", "disease", "gd"),
          ("gene", "gene", "gg")],
    NCORE=8,
    BANK=32768,     # dma_gather int16 row-index limit per table slice
)

NTYPES = ("drug", "gene", "disease")
SRC_NTYPES = ("drug", "gene")


# ---------------------------------------------------------------------------
# host-side preprocessing
# ---------------------------------------------------------------------------

def _pack_idx(stream):
    """int array (len % 128 == 0) -> dma_gather idx layout [128, len/16] int16:
    idx i at (i%16, i//16), replicated across the 8 groups of 16 partitions."""
    n = stream.size
    v = stream.astype(np.int16).reshape(n // 16, 16).T
    return np.tile(v, (8, 1))


def _banks(cfg, snt):
    """Gather-table bank slices for source ntype snt.
    Table rows: 0 = zeros, 1..N = nodes, N+1 = zeros.
    Returns list of (start_row, end_row, pad_row_relative)."""
    n = cfg["N"][snt]
    trows = n + 2
    if trows <= cfg["BANK"]:
        return [(0, trows, 0)]
    return [(0, cfg["BANK"], 0), (cfg["BANK"], trows, n + 1 - cfg["BANK"])]


def preprocess(cfg, inputs):
    ncore = cfg["NCORE"]
    shard = {nt: cfg["N"][nt] // ncore for nt in NTYPES}
    nw = {nt: -(-shard[nt] // P) for nt in NTYPES}

    S = dict(cfg=cfg, nw=nw, shard=shard, rels=[])
    percore = [dict() for _ in range(ncore)]

    for r, (snt, dnt, tag) in enumerate(cfg["RELS"]):
        src = np.asarray(inputs["e_" + tag + "_s"]).astype(np.int64)
        dst = np.asarray(inputs["e_" + tag + "_d"]).astype(np.int64)
        banks = _banks(cfg, snt)
        nbank = len(banks)
        NW = nw[dnt]
        dsh = shard[dnt]
        npad = NW * P

        core_of = dst // dsh
        deg_all = np.bincount(dst, minlength=cfg["N"][dnt]).astype(np.int64)

        orders = []
        perm_cols = np.zeros((ncore, NW, P), np.int32)
        invdeg_cols = np.zeros((ncore, NW, P), np.float32)
        dcnt = np.zeros((ncore, NW, P, nbank), np.int64)
        row_all = src + 1

        for c in range(ncore):
            deg_pad = np.zeros(npad, np.int64)
            deg_pad[:dsh] = deg_all[c * dsh:(c + 1) * dsh]
            dn = np.arange(npad)
            order = np.lexsort((dn, -deg_pad, dn // P))  # [npad] sorted dn per window
            orders.append(order)
            perm_cols[c] = (order % P).reshape(NW, P)
            iv = (1.0 / np.maximum(deg_pad[order], 1.0)).astype(np.float32)
            iv[order >= dsh] = 0.0                        # dummy dst slots
            invdeg_cols[c] = iv.reshape(NW, P)
            m = core_of == c
            ld = dst[m] - c * dsh
            for b, (b0, b1, _) in enumerate(banks):
                inb = (row_all[m] >= b0) & (row_all[m] < b1)
                cb = np.bincount(ld[inb], minlength=npad)
                dcnt[c, :, :, b] = cb[order].reshape(NW, P)

        # subgroup depths, common across cores; (w,b) block sizes % 128
        dq = np.zeros((NW, NSUB, nbank), np.int64)
        for q in range(NSUB):
            dq[:, q, :] = dcnt[:, :, q * SUBG:(q + 1) * SUBG, :].max(axis=(0, 2))
        for b in range(nbank):
            dq[:, NSUB - 1, b] += (-dq[:, :, b].sum(axis=1)) % (P // SUBG)

        # block layout: bank-major, window, subgroup
        blocks = []
        OFF = np.full((NW, NSUB, nbank), -1, np.int64)
        off = 0
        gathers = []
        for b in range(nbank):
            gstart, gslots = off, 0
            for w in range(NW):
                wslots = int(dq[w, :, b].sum()) * SUBG
                if wslots == 0:
                    continue
                if gslots + wslots > GCAP and gslots > 0:
                    gathers.append((b, gstart, gslots))
                    gstart, gslots = off, 0
                for q in range(NSUB):
                    if dq[w, q, b] > 0:
                        blocks.append((w, b, q, int(dq[w, q, b]), off))
                        OFF[w, q, b] = off
                        off += int(dq[w, q, b]) * SUBG
                gslots += wslots
            if gslots > 0:
                gathers.append((b, gstart, gslots))
        nslots = max(off, P)
        maxg = max((g[2] for g in gathers), default=P)

        for c in range(ncore):
            stream = np.zeros(nslots, np.int16)
            for (w, b, q, d, o) in blocks:
                stream[o:o + d * SUBG] = banks[b][2]
            m = core_of == c
            sm_row = row_all[m]
            ld = dst[m] - c * dsh
            bank_of = np.zeros(sm_row.size, np.int64)
            rel_row = sm_row.copy()
            for b, (b0, b1, _) in enumerate(banks):
                inb = (sm_row >= b0) & (sm_row < b1)
                bank_of[inb] = b
                rel_row[inb] = sm_row[inb] - b0
            dp_of = np.zeros(npad, np.int64)
            dp_of[orders[c]] = np.arange(npad)
            e_dp = dp_of[ld]
            e_w, e_dpw = e_dp // P, e_dp % P
            e_q, e_i = e_dpw // SUBG, e_dpw % SUBG
            # slot rank within (dst, bank)
            key = e_dp * nbank + bank_of
            so = np.argsort(key, kind="stable")
            ks = key[so]
            starts = np.r_[0, np.flatnonzero(np.diff(ks)) + 1]
            sizes = np.diff(np.r_[starts, ks.size])
            cum = np.arange(ks.size) - np.repeat(starts, sizes)
            e_j = np.empty(ks.size, np.int64)
            e_j[so] = cum
            d_arr = dq[e_w, e_q, bank_of]
            pos = OFF[e_w, e_q, bank_of] + e_i * d_arr + e_j
            assert (pos >= 0).all() and (e_j < d_arr).all()
            stream[pos] = rel_row.astype(np.int16)
            percore[c][f"idx_{tag}"] = _pack_idx(stream)
            percore[c][f"perm_{tag}"] = np.ascontiguousarray(
                perm_cols[c].astype(np.float32).T)
            percore[c][f"invdeg_{tag}"] = np.ascontiguousarray(
                invdeg_cols[c].astype(np.float32).T)

        S["rels"].append(dict(r=r, snt=snt, dnt=dnt, tag=tag, NW=NW,
                              banks=banks, blocks=blocks, gathers=gathers,
                              nslots=nslots, maxg=maxg))

    for nt in NTYPES:
        x = np.asarray(inputs["x_" + nt])
        for c in range(ncore):
            sh = shard[nt]
            percore[c][f"xT_{nt}"] = np.ascontiguousarray(
                x[c * sh:(c + 1) * sh].T).astype(BF16)

    com = dict()
    for nt in NTYPES:
        com[f"We_{nt}"] = np.asarray(inputs["We_" + nt]).astype(BF16)
        com[f"be_{nt}"] = np.asarray(inputs["be_" + nt]).astype(
            np.float32).reshape(-1, 1)
    for l in range(3):
        com[f"W{l}"] = np.asarray(inputs[f"W{l}"]).astype(BF16)
        com[f"L{l}"] = np.asarray(inputs[f"L{l}"]).astype(BF16)
        com[f"b{l}"] = np.asarray(inputs[f"b{l}"]).astype(np.float32).reshape(-1, 1)
    com["iota"] = np.tile(np.arange(P, dtype=np.float32), (P, 1))
    for c in range(ncore):
        percore[c].update(com)
    return S, percore


# ---------------------------------------------------------------------------
# device program
# ---------------------------------------------------------------------------

def build(S):
    cfg = S["cfg"]
    ncore = cfg["NCORE"]
    nw, shard = S["nw"], S["shard"]
    DH, DOUT = cfg["D_H"], cfg["D_OUT"]
    NREL = len(cfg["RELS"])
    nsh_tot = sum(shard.values())
    maxg_all = max(R["maxg"] for R in S["rels"])
    maxw_cols = max(nw[nt] for nt in NTYPES) * P

    nc = bacc.Bacc("TRN2", target_bir_lowering=False, debug=False,
                   num_devices=ncore)

    par = {}
    for nt in NTYPES:
        par[f"xT_{nt}"] = nc.declare_dram_parameter(
            f"xT_{nt}", [cfg["MOD"][nt], shard[nt]], mybir.dt.bfloat16, False)
        par[f"We_{nt}"] = nc.declare_dram_parameter(
            f"We_{nt}", [cfg["MOD"][nt], cfg["D_IN"]], mybir.dt.bfloat16, False)
        par[f"be_{nt}"] = nc.declare_dram_parameter(
            f"be_{nt}", [cfg["D_IN"], 1], mybir.dt.float32, False)
    for l in range(3):
        od = DOUT if l == 2 else DH
        par[f"W{l}"] = nc.declare_dram_parameter(
            f"W{l}", [NREL, DH, od], mybir.dt.bfloat16, False)
        par[f"L{l}"] = nc.declare_dram_parameter(
            f"L{l}", [DH, od], mybir.dt.bfloat16, False)
        par[f"b{l}"] = nc.declare_dram_parameter(
            f"b{l}", [od, 1], mybir.dt.float32, False)
    par["iota"] = nc.declare_dram_parameter("iota", [P, P], mybir.dt.float32, False)
    for R in S["rels"]:
        tg = R["tag"]
        par[f"idx_{tg}"] = nc.declare_dram_parameter(
            f"idx_{tg}", [P, R["nslots"] // 16], mybir.dt.int16, False)
        par[f"perm_{tg}"] = nc.declare_dram_parameter(
            f"perm_{tg}", [P, R["NW"]], mybir.dt.float32, False)
        par[f"invdeg_{tg}"] = nc.declare_dram_parameter(
            f"invdeg_{tg}", [P, R["NW"]], mybir.dt.float32, False)
    out_par = nc.declare_dram_parameter("out", [nsh_tot, DOUT],
                                        mybir.dt.float32, True)

    agin, tabs = {}, {}
    for l in range(3):
        for nt in SRC_NTYPES:
            agin[(l, nt)] = nc.dram_tensor(
                f"agin{l}_{nt}", [shard[nt], DH], mybir.dt.bfloat16)
            tabs[(l, nt)] = nc.dram_tensor(
                f"tab{l}_{nt}", [cfg["N"][nt] + 2, DH], mybir.dt.bfloat16,
                addr_space="Shared")

    with ExitStack() as ctx:
        tc = ctx.enter_context(tile.TileContext(nc))
        nc.gpsimd.load_library(library_config.mlp)

        const = ctx.enter_context(tc.tile_pool(name="const", bufs=1))
        persist = ctx.enter_context(tc.tile_pool(name="persist", bufs=1))
        gpool = ctx.enter_context(tc.tile_pool(name="gpool", bufs=3))
        ipool = ctx.enter_context(tc.tile_pool(name="ipool", bufs=4))
        xpool = ctx.enter_context(tc.tile_pool(name="xpool", bufs=2))
        wpool = ctx.enter_context(tc.tile_pool(name="wpool", bufs=4))
        pst = ctx.enter_context(tc.tile_pool(name="pst", bufs=2, space="PSUM"))
        ps1 = ctx.enter_context(tc.tile_pool(name="ps1", bufs=2, space="PSUM"))
        ps2 = ctx.enter_context(tc.tile_pool(name="ps2", bufs=2, space="PSUM"))
        psE = ctx.enter_context(tc.tile_pool(name="psE", bufs=2, space="PSUM"))

        sb_iota = const.tile([P, P], mybir.dt.float32)
        nc.sync.dma_start(sb_iota[:], par["iota"][:])
        identity = const.tile([P, P], mybir.dt.float32)
        from concourse.masks import make_identity
        make_identity(nc, identity[:])
        identity16 = const.tile([P, P], mybir.dt.bfloat16)
        nc.vector.tensor_copy(identity16[:], identity[:])

        sb_W, sb_L, sb_b = {}, {}, {}
        for l in range(3):
            od = DOUT if l == 2 else DH
            t = const.tile([DH, NREL, od], mybir.dt.bfloat16, tag=f"W{l}")
            nc.sync.dma_start(t[:], par[f"W{l}"][:].rearrange("r k o -> k r o"))
            sb_W[l] = t
            sb_L[l] = const.tile([DH, od], mybir.dt.bfloat16, tag=f"L{l}", name=f"L{l}")
            nc.sync.dma_start(sb_L[l][:], par[f"L{l}"][:])
            sb_b[l] = const.tile([od, 1], mybir.dt.float32, tag=f"b{l}", name=f"b{l}")
            nc.sync.dma_start(sb_b[l][:], par[f"b{l}"][:])
        sb_meta = {}
        for R in S["rels"]:
            tg = R["tag"]
            pm = const.tile([P, R["NW"]], mybir.dt.float32, tag=f"pm_{tg}")
            nc.sync.dma_start(pm[:], par[f"perm_{tg}"][:])
            iv = const.tile([P, R["NW"]], mybir.dt.float32, tag=f"iv_{tg}")
            nc.sync.dma_start(iv[:], par[f"invdeg_{tg}"][:])
            sb_meta[tg] = (pm, iv)

        zrow = const.tile([1, DH], mybir.dt.bfloat16)
        nc.vector.memset(zrow[:], 0.0)
        for l in range(3):
            for nt in SRC_NTYPES:
                n = cfg["N"][nt]
                nc.sync.dma_start(tabs[(l, nt)][0:1, :], zrow[:])
                nc.sync.dma_start(tabs[(l, nt)][n + 1:n + 2, :], zrow[:])

        hT = [persist.tile([DH, nsh_tot], mybir.dt.bfloat16, tag=f"hT{i}",
                           name=f"hT{i}")
              for i in range(2)]
        nt_off, o = {}, 0
        for nt in NTYPES:
            nt_off[nt] = o
            o += shard[nt]
        agg = persist.tile([DH, nsh_tot], mybir.dt.float32, tag="agg")
        praw = persist.tile([DH, maxw_cols], mybir.dt.float32, tag="praw")

        def emit_embedding():
            for nt in NTYPES:
                mod, sh = cfg["MOD"][nt], shard[nt]
                kt = mod // P
                sb_we = xpool.tile([P, 8, cfg["D_IN"]], mybir.dt.bfloat16, tag="we")
                nc.sync.dma_start(
                    sb_we[:, :kt, :],
                    par[f"We_{nt}"][:].rearrange("(k p) f -> p k f", p=P))
                sb_be = wpool.tile([cfg["D_IN"], 1], mybir.dt.float32, tag="be")
                nc.sync.dma_start(sb_be[:], par[f"be_{nt}"][:])
                for n0 in range(0, sh, 512):
                    n1 = min(n0 + 512, sh)
                    cols = n1 - n0
                    xt = xpool.tile([P, 8, 512], mybir.dt.bfloat16, tag="xt")
                    nc.sync.dma_start(
                        xt[:, :kt, :cols],
                        par[f"xT_{nt}"][:].rearrange(
                            "(k p) n -> p k n", p=P)[:, :, n0:n1])
                    pe = psE.tile([P, 512], mybir.dt.float32, tag="emb")
                    for k in range(kt):
                        nc.tensor.matmul(pe[:, :cols], sb_we[:, k, :],
                                         xt[:, k, :cols],
                                         start=(k == 0), stop=(k == kt - 1))
                    nc.scalar.activation(
                        hT[0][:, nt_off[nt] + n0:nt_off[nt] + n1], pe[:, :cols],
                        mybir.ActivationFunctionType.Identity, bias=sb_be[:])

        def emit_ag(l):
            for nt in SRC_NTYPES:
                sh = shard[nt]
                for w0 in range(0, sh, P):
                    cols = min(P, sh - w0)
                    src = hT[l % 2][:, nt_off[nt] + w0:nt_off[nt] + w0 + cols]
                    pt = pst.tile([P, P], mybir.dt.bfloat16, tag="tp", name="pt16")
                    nc.tensor.transpose(pt[:cols, :DH], src, identity16[:])
                    stg = wpool.tile([P, DH], mybir.dt.bfloat16, tag="agstg")
                    nc.vector.tensor_copy(stg[:cols, :], pt[:cols, :DH])
                    nc.sync.dma_start(agin[(l, nt)][w0:w0 + cols, :],
                                      stg[:cols, :])
                nc.gpsimd.collective_compute(
                    "AllGather", mybir.AluOpType.bypass,
                    replica_groups=[list(range(ncore))],
                    ins=[agin[(l, nt)][:]],
                    outs=[tabs[(l, nt)][1:cfg["N"][nt] + 1]],
                )

        gq_counter = [0]

        def emit_layer(l):
            od = DOUT if l == 2 else DH
            nc.vector.memset(agg[:od, :], 0.0)
            for R in S["rels"]:
                tg, snt, dnt, r, NW = R["tag"], R["snt"], R["dnt"], R["r"], R["NW"]
                tab = tabs[(l, snt)]
                pm, iv = sb_meta[tg]
                bank_written = np.zeros((NW, NSUB), bool)
                # blocks grouped per gather
                blk_by_g = {gi: [] for gi in range(len(R["gathers"]))}
                for blk in R["blocks"]:
                    for gi, (b, goff, gslots) in enumerate(R["gathers"]):
                        if goff <= blk[4] < goff + gslots:
                            blk_by_g[gi].append(blk)
                            break
                for gi, (b, goff, gslots) in enumerate(R["gathers"]):
                    sbi = ipool.tile([P, maxg_all // 16], mybir.dt.int16,
                                     tag="idx")
                    nc.sync.dma_start(
                        sbi[:, :gslots // 16],
                        par[f"idx_{tg}"][:, goff // 16:(goff + gslots) // 16])
                    gt = gpool.tile([P, 1, maxg_all], mybir.dt.bfloat16,
                                    tag="gat")
                    b0, b1, _ = R["banks"][b]
                    nc.gpsimd.dma_gather(
                        out_ap=gt[:, :, :gslots], in_ap=tab[b0:b1],
                        idxs_ap=sbi[:, :gslots // 16],
                        num_idxs=gslots, num_idxs_reg=gslots,
                        elem_size=DH, transpose=True,
                        single_packet=(gslots <= 992))
                    for (w, bb, q, d, off) in blk_by_g[gi]:
                        loc = off - goff
                        view = gt[:, 0, loc:loc + d * SUBG].rearrange(
                            "p (n d) -> p n d", d=d)
                        cols = slice(w * P + q * SUBG, w * P + (q + 1) * SUBG)
                        if not bank_written[w, q]:
                            nc.vector.tensor_reduce(
                                praw[:, cols], view, axis=mybir.AxisListType.X,
                                op=mybir.AluOpType.add)
                            bank_written[w, q] = True
                        else:
                            tmp = wpool.tile([P, SUBG], mybir.dt.float32,
                                             tag="rtmp")
                            nc.vector.tensor_reduce(
                                tmp[:], view, axis=mybir.AxisListType.X,
                                op=mybir.AluOpType.add)
                            nc.vector.tensor_add(praw[:, cols], praw[:, cols],
                                                 tmp[:])
                for w in range(NW):
                    for q in range(NSUB):
                        if not bank_written[w, q]:
                            nc.vector.memset(
                                praw[:, w * P + q * SUBG:w * P + (q + 1) * SUBG],
                                0.0)
                for w in range(NW):
                    praw16 = wpool.tile([P, P], mybir.dt.bfloat16, tag="praw16")
                    nc.vector.tensor_copy(praw16[:], praw[:, w * P:(w + 1) * P])
                    Sp = wpool.tile([P, P], mybir.dt.bfloat16, tag="Sperm")
                    nc.vector.tensor_scalar(
                        Sp[:], sb_iota[:], pm[:, w:w + 1], iv[:, w:w + 1],
                        op0=mybir.AluOpType.is_equal, op1=mybir.AluOpType.mult)
                    p1 = ps1.tile([P, DH], mybir.dt.float32, tag="out1")
                    nc.tensor.matmul(p1[:, :od], praw16[:], sb_W[l][:, r, :],
                                     start=True, stop=True)
                    o1 = wpool.tile([P, DH], mybir.dt.bfloat16, tag="o1")
                    nc.vector.tensor_copy(o1[:, :od], p1[:, :od])
                    p2 = ps2.tile([P, P], mybir.dt.float32, tag="out2")
                    nc.tensor.matmul(p2[:od, :], o1[:, :od], Sp[:],
                                     start=True, stop=True)
                    cs = nt_off[dnt] + w * P
                    ce = min(cs + P, nt_off[dnt] + shard[dnt])
                    nc.vector.tensor_add(agg[:od, cs:ce], agg[:od, cs:ce],
                                         p2[:od, :ce - cs])
            for nt in NTYPES:
                sh = shard[nt]
                for w0 in range(0, sh, P):
                    cols = min(P, sh - w0)
                    cs = nt_off[nt] + w0
                    p2 = ps2.tile([P, P], mybir.dt.float32, tag="out2")
                    nc.tensor.matmul(p2[:od, :cols], sb_L[l][:],
                                     hT[l % 2][:, cs:cs + cols],
                                     start=True, stop=True)
                    nc.vector.tensor_add(agg[:od, cs:cs + cols],
                                         agg[:od, cs:cs + cols],
                                         p2[:od, :cols])
                    if l < 2:
                        nc.scalar.activation(
                            hT[(l + 1) % 2][:od, cs:cs + cols],
                            agg[:od, cs:cs + cols],
                            mybir.ActivationFunctionType.Relu, bias=sb_b[l][:])
                    else:
                        fin = wpool.tile([P, P], mybir.dt.float32, tag="fin")
                        nc.scalar.activation(
                            fin[:od, :cols], agg[:od, cs:cs + cols],
                            mybir.ActivationFunctionType.Identity,
                            bias=sb_b[l][:])
                        pt = pst.tile([P, P], mybir.dt.float32, tag="tp")
                        nc.tensor.transpose(pt[:cols, :od], fin[:od, :cols],
                                            identity[:od, :od])
                        stg = wpool.tile([P, DOUT], mybir.dt.float32, tag="ostg")
                        nc.vector.tensor_copy(stg[:cols, :], pt[:cols, :od])
                        nc.sync.dma_start(out_par[cs:cs + cols, :],
                                          stg[:cols, :])

        emit_embedding()
        emit_ag(0)
        emit_layer(0)
        emit_ag(1)
        emit_layer(1)
        emit_ag(2)
        emit_layer(2)

    nc.compile()
    return nc


# ---------------------------------------------------------------------------
# entry point
# ---------------------------------------------------------------------------

def _install_ntff_hook():
    if "antenv.axon_hooks" in sys.modules:
        return
    mod = types.ModuleType("antenv.axon_hooks")
    mod._hook = None
    mod.set_axon_ntff_profile_hook = lambda h: setattr(mod, "_hook", h)
    mod.get_axon_ntff_profile_hook = lambda: mod._hook
    sys.modules["antenv.axon_hooks"] = mod
    try:
        import antenv
        antenv.axon_hooks = mod
        from trn_agent_boot.trn_boot import _ntff_profile_via_ctypes
        hook = _ntff_profile_via_ctypes("/opt/axon/libaxon_pjrt.so")
        if hook is not None:
            mod.set_axon_ntff_profile_hook(hook)
    except Exception:
        pass


def run(inputs, cfg=CFG, trace=False, tmpdir=None):
    S, percore = preprocess(cfg, inputs)
    nc = build(S)
    _install_ntff_hook()
    from concourse import bass_utils
    bass_utils.upload_artifacts = lambda d: d
    res = bass_utils.run_bass_kernel_spmd(
        nc, percore, list(range(cfg["NCORE"])), trace=trace, tmpdir=tmpdir,
        trace_cores=[0] if trace else None)
    ncore = cfg["NCORE"]
    shard = {nt: cfg["N"][nt] // ncore for nt in NTYPES}
    outs = []
    o = 0
    for nt in NTYPES:
        parts = [res.results[c]["out"][o:o + shard[nt]] for c in range(ncore)]
        outs.append(np.concatenate(parts, 0))
        o += shard[nt]
    full = np.concatenate(outs, 0).astype(np.float32)
    run.last_exec_time_ns = res.exec_time_ns
    return full


def kernel(**inputs):
    return run(inputs)



# revision 13
# speedup vs baseline: 1.5367x; 1.5367x over previous
"""Trainium2 Bass kernel for nn_BaseRGCNHetero (3-layer heterogeneous RGCN).

Strategy (8 NeuronCores, SPMD):
  - Destination-shard the nodes: core c owns rows [c*N/8, (c+1)*N/8) of every
    node type; all edges whose dst is in the shard are processed there, so
    per-relation aggregates need no cross-core reduction.
  - Aggregate-first algebra: agg[dst] = (sum_{e->dst} h[src]) @ W_r * inv_deg,
    sharing one bf16 DRAM gather table per source ntype (drug, gene) per
    layer.  After each layer the drug/gene h-shards are AllGathered (bf16)
    into the next layer's tables; each AllGather is issued as soon as its
    ntype's windows finish so the wire time overlaps the remaining gathers.
  - Edge slot stream per relation: 128-dst windows in natural order, one
    chunk per (window, src bank), slots sorted by src row and padded to a
    multiple of 128 (pad slots point at an all-zero table row).  Non-
    transposed dma_gather pulls h[src] rows node-major ([slot, feat] blocks);
    gathers round-robin over 4 SWDGE queues so descriptor generation runs on
    all four GpSimd Q7 core pairs concurrently (3.3x single-queue, and the
    non-transpose path avoids the xbar that makes concurrent transposed
    gathers corrupt each other).
  - Segment sums on TensorE: per 128-slot block, VectorE builds a one-hot
    seg matrix seg[slot, dst] = (dstcol[slot] == dst) * inv_deg[slot] from
    host-streamed per-block columns; matmul(gt_block^T @ seg) accumulates
    agg[feat, dst] for the window in PSUM across the window's blocks.
  - Per dst window: agg -> bf16 praw, then one matmul per relation (W_r) plus
    the self-loop h @ L accumulate in a single PSUM bank; bias (+relu) is a
    fused ScalarE activation per window.
"""
import sys
import types
import numpy as np
import ml_dtypes
from contextlib import ExitStack

import concourse.bass as bass
import concourse.bacc as bacc
import concourse.tile as tile
from concourse import mybir, library_config

BF16 = ml_dtypes.bfloat16
P = 128
NQ = 4             # SWDGE queues (gather descriptor-gen parallelism)

CFG = dict(
    N={"drug": 20000, "gene": 50000, "disease": 10000},
    MOD={"drug": 1024, "gene": 768, "disease": 512},
    D_IN=128, D_H=128, D_OUT=64,
    RELS=[("drug", "disease", "dd"), ("drug", "drug", "ddr"),
          ("drug", "gene", "dg"), ("gene", "disease", "gd"),
          ("gene", "gene", "gg")],
    NCORE=8,
    BANK=32768,     # dma_gather int16 row-index limit per table slice
)

NTYPES = ("drug", "gene", "disease")
SRC_NTYPES = ("drug", "gene")
# layer processing phases: dst ntype -> relations feeding it (tags)
PHASES = [("gene", ["dg", "gg"]), ("drug", ["ddr"]), ("disease", ["dd", "gd"])]


# ---------------------------------------------------------------------------
# host-side preprocessing
# ---------------------------------------------------------------------------

def _pack_idx(stream):
    """int array (len % 128 == 0) -> dma_gather idx layout [128, len/16] int16:
    idx i at (i%16, i//16), replicated across the 8 groups of 16 partitions."""
    n = stream.size
    v = stream.astype(np.int16).reshape(n // 16, 16).T
    return np.tile(v, (8, 1))


def _banks(cfg, snt):
    """Gather-table bank slices for source ntype snt.
    Table rows: 0 = zeros, 1..N = nodes, N+1 = zeros.
    Returns list of (start_row, end_row, pad_row_relative)."""
    n = cfg["N"][snt]
    trows = n + 2
    if trows <= cfg["BANK"]:
        return [(0, trows, 0)]
    return [(0, cfg["BANK"], 0), (cfg["BANK"], trows, n + 1 - cfg["BANK"])]


def preprocess(cfg, inputs):
    ncore = cfg["NCORE"]
    shard = {nt: cfg["N"][nt] // ncore for nt in NTYPES}
    nw = {nt: -(-shard[nt] // P) for nt in NTYPES}

    S = dict(cfg=cfg, nw=nw, shard=shard, rels=[])
    percore = [dict() for _ in range(ncore)]

    for r, (snt, dnt, tag) in enumerate(cfg["RELS"]):
        src = np.asarray(inputs["e_" + tag + "_s"]).astype(np.int64)
        dst = np.asarray(inputs["e_" + tag + "_d"]).astype(np.int64)
        banks = _banks(cfg, snt)
        nbank = len(banks)
        NW = nw[dnt]
        dsh = shard[dnt]

        core_of = dst // dsh
        deg_all = np.bincount(dst, minlength=cfg["N"][dnt]).astype(np.float32)
        inv_deg = 1.0 / np.maximum(deg_all, 1.0)

        row_all = src + 1
        bank_of = (row_all >= cfg["BANK"]).astype(np.int64) if nbank == 2 \
            else np.zeros(row_all.size, np.int64)

        # per-core per-(window, bank) edge counts -> shared block counts
        cnt = np.zeros((ncore, NW, nbank), np.int64)
        ld_all = dst - core_of * dsh
        w_all = ld_all // P
        for c in range(ncore):
            m = core_of == c
            key = w_all[m] * nbank + bank_of[m]
            cnt[c] = np.bincount(key, minlength=NW * nbank).reshape(NW, nbank)
        nblk = -(-cnt.max(axis=0) // P)          # [NW, nbank] shared
        slots_wb = nblk * P
        off_wb = np.zeros((NW, nbank), np.int64)
        gathers = []                              # (bank, off, slots, nblk, segoff)
        off = 0
        segoff = 0
        for w in range(NW):
            for b in range(nbank):
                if nblk[w, b] == 0:
                    continue
                off_wb[w, b] = off
                gathers.append((w, b, int(off), int(slots_wb[w, b]),
                                int(nblk[w, b]), int(segoff)))
                off += int(slots_wb[w, b])
                segoff += int(nblk[w, b])
        nslots = max(off, P)
        nblk_tot = max(segoff, 1)
        maxblk = int(nblk.max()) if nblk.size else 1

        for c in range(ncore):
            stream = np.zeros(nslots, np.int16)
            segc = np.zeros((nblk_tot, P, 2), np.float32)
            for w, b, o, sl, nb, so in gathers:
                stream[o:o + sl] = banks[b][2]
            m = core_of == c
            e_row = row_all[m] - np.array([bk[0] for bk in banks])[bank_of[m]]
            e_b = bank_of[m]
            e_ld = ld_all[m]
            e_w = e_ld // P
            e_dl = e_ld % P
            e_iv = inv_deg[dst[m]]
            order = np.lexsort((e_row, e_b, e_w))
            key = (e_w * nbank + e_b)[order]
            starts = np.r_[0, np.flatnonzero(np.diff(key)) + 1]
            sizes = np.diff(np.r_[starts, key.size])
            rank = np.arange(key.size) - np.repeat(starts, sizes)
            pos = off_wb[e_w[order], e_b[order]] + rank
            stream[pos] = e_row[order].astype(np.int16)
            blk = pos // P
            sl = pos % P
            segc[blk, sl, 0] = e_dl[order]
            segc[blk, sl, 1] = e_iv[order]
            percore[c][f"idx_{tag}"] = _pack_idx(stream)
            percore[c][f"segc_{tag}"] = np.ascontiguousarray(
                segc.transpose(1, 0, 2))

        S["rels"].append(dict(r=r, snt=snt, dnt=dnt, tag=tag, NW=NW,
                              banks=banks, gathers=gathers, nslots=nslots,
                              nblk_tot=nblk_tot, maxblk=maxblk))

    for nt in NTYPES:
        x = np.asarray(inputs["x_" + nt])
        for c in range(ncore):
            sh = shard[nt]
            percore[c][f"xT_{nt}"] = np.ascontiguousarray(
                x[c * sh:(c + 1) * sh].T).astype(BF16)

    com = dict()
    for nt in NTYPES:
        com[f"We_{nt}"] = np.asarray(inputs["We_" + nt]).astype(BF16)
        com[f"be_{nt}"] = np.asarray(inputs["be_" + nt]).astype(
            np.float32).reshape(-1, 1)
    for l in range(3):
        com[f"W{l}"] = np.asarray(inputs[f"W{l}"]).astype(BF16)
        com[f"L{l}"] = np.asarray(inputs[f"L{l}"]).astype(BF16)
        com[f"b{l}"] = np.asarray(inputs[f"b{l}"]).astype(np.float32).reshape(-1, 1)
    com["iota"] = np.tile(np.arange(P, dtype=np.float32), (P, 1)).astype(BF16)
    for c in range(ncore):
        percore[c].update(com)
    return S, percore


# ---------------------------------------------------------------------------
# device program
# ---------------------------------------------------------------------------

def build(S):
    cfg = S["cfg"]
    ncore = cfg["NCORE"]
    nw, shard = S["nw"], S["shard"]
    DH, DOUT = cfg["D_H"], cfg["D_OUT"]
    NREL = len(cfg["RELS"])
    nsh_tot = sum(shard.values())
    rel_by_tag = {R["tag"]: R for R in S["rels"]}
    maxblk_all = max(R["maxblk"] for R in S["rels"])
    # praw16 column offsets per phase (buffers reused across phases)
    praw_off = {}
    praw_cols = 0
    for dnt, tags in PHASES:
        o = 0
        for tg in tags:
            praw_off[tg] = o
            o += nw[dnt] * P
        praw_cols = max(praw_cols, o)

    nc = bacc.Bacc("TRN2", target_bir_lowering=False, debug=False,
                   num_devices=ncore, num_swdge_queues=NQ)

    par = {}
    for nt in NTYPES:
        par[f"xT_{nt}"] = nc.declare_dram_parameter(
            f"xT_{nt}", [cfg["MOD"][nt], shard[nt]], mybir.dt.bfloat16, False)
        par[f"We_{nt}"] = nc.declare_dram_parameter(
            f"We_{nt}", [cfg["MOD"][nt], cfg["D_IN"]], mybir.dt.bfloat16, False)
        par[f"be_{nt}"] = nc.declare_dram_parameter(
            f"be_{nt}", [cfg["D_IN"], 1], mybir.dt.float32, False)
    for l in range(3):
        od = DOUT if l == 2 else DH
        par[f"W{l}"] = nc.declare_dram_parameter(
            f"W{l}", [NREL, DH, od], mybir.dt.bfloat16, False)
        par[f"L{l}"] = nc.declare_dram_parameter(
            f"L{l}", [DH, od], mybir.dt.bfloat16, False)
        par[f"b{l}"] = nc.declare_dram_parameter(
            f"b{l}", [od, 1], mybir.dt.float32, False)
    par["iota"] = nc.declare_dram_parameter("iota", [P, P], mybir.dt.bfloat16,
                                            False)
    for R in S["rels"]:
        tg = R["tag"]
        par[f"idx_{tg}"] = nc.declare_dram_parameter(
            f"idx_{tg}", [P, R["nslots"] // 16], mybir.dt.int16, False)
        par[f"segc_{tg}"] = nc.declare_dram_parameter(
            f"segc_{tg}", [P, R["nblk_tot"], 2], mybir.dt.float32, False)
    out_par = nc.declare_dram_parameter("out", [nsh_tot, DOUT],
                                        mybir.dt.float32, True)

    agin, tabs = {}, {}
    for l in range(3):
        for nt in SRC_NTYPES:
            agin[(l, nt)] = nc.dram_tensor(
                f"agin{l}_{nt}", [shard[nt], DH], mybir.dt.bfloat16)
            tabs[(l, nt)] = nc.dram_tensor(
                f"tab{l}_{nt}", [cfg["N"][nt] + 2, DH], mybir.dt.bfloat16,
                addr_space="Shared")

    with ExitStack() as ctx:
        tc = ctx.enter_context(tile.TileContext(nc))
        nc.gpsimd.load_library(library_config.mlp)

        const = ctx.enter_context(tc.tile_pool(name="const", bufs=1))
        persist = ctx.enter_context(tc.tile_pool(name="persist", bufs=1))
        gpool = ctx.enter_context(tc.tile_pool(name="gpool", bufs=6))
        ipool = ctx.enter_context(tc.tile_pool(name="ipool", bufs=8))
        xpool = ctx.enter_context(tc.tile_pool(name="xpool", bufs=2))
        wpool = ctx.enter_context(tc.tile_pool(name="wpool", bufs=4))
        spool = ctx.enter_context(tc.tile_pool(name="spool", bufs=6))
        segp = ctx.enter_context(tc.tile_pool(name="segp", bufs=6))
        pst = ctx.enter_context(tc.tile_pool(name="pst", bufs=2, space="PSUM"))
        psA = ctx.enter_context(tc.tile_pool(name="psA", bufs=2, space="PSUM"))
        psB = ctx.enter_context(tc.tile_pool(name="psB", bufs=2, space="PSUM"))
        psE = ctx.enter_context(tc.tile_pool(name="psE", bufs=2, space="PSUM"))

        identity = const.tile([P, P], mybir.dt.float32)
        from concourse.masks import make_identity
        make_identity(nc, identity[:])
        identity16 = const.tile([P, P], mybir.dt.bfloat16)
        nc.vector.tensor_copy(identity16[:], identity[:])
        sb_iota = const.tile([P, P], mybir.dt.bfloat16, tag="iota")
        nc.sync.dma_start(sb_iota[:], par["iota"][:])

        sb_W, sb_L, sb_b = {}, {}, {}
        for l in range(3):
            od = DOUT if l == 2 else DH
            t = const.tile([DH, NREL, od], mybir.dt.bfloat16, tag=f"W{l}")
            nc.sync.dma_start(t[:], par[f"W{l}"][:].rearrange("r k o -> k r o"))
            sb_W[l] = t
            sb_L[l] = const.tile([DH, od], mybir.dt.bfloat16, tag=f"L{l}",
                                 name=f"L{l}")
            nc.sync.dma_start(sb_L[l][:], par[f"L{l}"][:])
            sb_b[l] = const.tile([od, 1], mybir.dt.float32, tag=f"b{l}",
                                 name=f"b{l}")
            nc.sync.dma_start(sb_b[l][:], par[f"b{l}"][:])

        zrow = const.tile([1, DH], mybir.dt.bfloat16)
        nc.vector.memset(zrow[:], 0.0)
        for l in range(3):
            for nt in SRC_NTYPES:
                n = cfg["N"][nt]
                nc.sync.dma_start(tabs[(l, nt)][0:1, :], zrow[:])
                nc.sync.dma_start(tabs[(l, nt)][n + 1:n + 2, :], zrow[:])

        hT = [persist.tile([DH, nsh_tot], mybir.dt.bfloat16, tag=f"hT{i}",
                           name=f"hT{i}")
              for i in range(2)]
        nt_off, o = {}, 0
        for nt in NTYPES:
            nt_off[nt] = o
            o += shard[nt]
        praw = persist.tile([DH, praw_cols], mybir.dt.bfloat16, tag="praw")

        gq_counter = [0]

        def emit_embedding(nt):
            mod, sh = cfg["MOD"][nt], shard[nt]
            kt = mod // P
            sb_we = xpool.tile([P, 8, cfg["D_IN"]], mybir.dt.bfloat16, tag="we")
            nc.sync.dma_start(
                sb_we[:, :kt, :],
                par[f"We_{nt}"][:].rearrange("(k p) f -> p k f", p=P))
            sb_be = wpool.tile([cfg["D_IN"], 1], mybir.dt.float32, tag="be")
            nc.sync.dma_start(sb_be[:], par[f"be_{nt}"][:])
            for n0 in range(0, sh, 512):
                n1 = min(n0 + 512, sh)
                cols = n1 - n0
                xt = xpool.tile([P, 8, 512], mybir.dt.bfloat16, tag="xt")
                nc.sync.dma_start(
                    xt[:, :kt, :cols],
                    par[f"xT_{nt}"][:].rearrange(
                        "(k p) n -> p k n", p=P)[:, :, n0:n1])
                pe = psE.tile([P, 512], mybir.dt.float32, tag="emb")
                for k in range(kt):
                    nc.tensor.matmul(pe[:, :cols], sb_we[:, k, :],
                                     xt[:, k, :cols],
                                     start=(k == 0), stop=(k == kt - 1))
                nc.scalar.activation(
                    hT[0][:, nt_off[nt] + n0:nt_off[nt] + n1], pe[:, :cols],
                    mybir.ActivationFunctionType.Identity, bias=sb_be[:])

        def stage_ag_window(l, nt, w0, cols):
            """Transpose one hT[l] window of ntype nt into the AllGather
            staging buffer."""
            src = hT[l % 2][:, nt_off[nt] + w0:nt_off[nt] + w0 + cols]
            pt = pst.tile([P, P], mybir.dt.bfloat16, tag="tp", name="pt16")
            nc.tensor.transpose(pt[:cols, :DH], src, identity16[:])
            stg = wpool.tile([P, DH], mybir.dt.bfloat16, tag="agstg")
            nc.vector.tensor_copy(stg[:cols, :], pt[:cols, :DH])
            nc.sync.dma_start(agin[(l, nt)][w0:w0 + cols, :], stg[:cols, :])

        def emit_ag_collective(l, nt):
            nc.gpsimd.collective_compute(
                "AllGather", mybir.AluOpType.bypass,
                replica_groups=[list(range(ncore))],
                ins=[agin[(l, nt)][:]],
                outs=[tabs[(l, nt)][1:cfg["N"][nt] + 1]],
            )

        def emit_ag(l, nt):
            for w0 in range(0, shard[nt], P):
                stage_ag_window(l, nt, w0, min(P, shard[nt] - w0))
            emit_ag_collective(l, nt)

        def emit_window(l, dnt, tags, w):
            """One dst window: per-relation gathers + seg matmuls into PSUM
            agg -> praw, then W_r matmuls + self-loop + activation."""
            od = DOUT if l == 2 else DH
            sh = shard[dnt]
            cs = nt_off[dnt] + w * P
            ce = min(cs + P, nt_off[dnt] + sh)
            cols = ce - cs
            live = []
            for tg in tags:
                R = rel_by_tag[tg]
                gs = [g for g in R["gathers"] if g[0] == w]
                nmm = sum(g[4] for g in gs)
                if nmm == 0:
                    nc.vector.memset(
                        praw[:, praw_off[tg] + w * P:
                         praw_off[tg] + (w + 1) * P], 0.0)
                    continue
                pa = psA.tile([P, P], mybir.dt.float32, tag="agg")
                mm = 0
                for (_, b, soff, slots, nb, segoff) in gs:
                    sbi = ipool.tile([P, maxblk_all * P // 16],
                                     mybir.dt.int16, tag="idx")
                    nc.sync.dma_start(
                        sbi[:, :slots // 16],
                        par[f"idx_{tg}"][:, soff // 16:(soff + slots) // 16])
                    gt = gpool.tile([P, maxblk_all, P], mybir.dt.bfloat16,
                                    tag="gat")
                    b0, b1, _ = R["banks"][b]
                    q = gq_counter[0] % NQ
                    gq_counter[0] += 1
                    nc.gpsimd.dma_gather(
                        out_ap=gt[:, :nb, :], in_ap=tabs[(l, R["snt"])][b0:b1],
                        idxs_ap=sbi[:, :slots // 16],
                        num_idxs=slots, num_idxs_reg=slots,
                        elem_size=DH, transpose=False, single_packet=False,
                        queue_num=q)
                    sc = spool.tile([P, maxblk_all, 2], mybir.dt.float32,
                                    tag="sc")
                    nc.sync.dma_start(
                        sc[:, :nb, :],
                        par[f"segc_{tg}"][:, segoff:segoff + nb, :])
                    for k in range(nb):
                        seg = segp.tile([P, P], mybir.dt.bfloat16, tag="seg")
                        nc.vector.tensor_scalar(
                            seg[:], sb_iota[:], sc[:, k, 0:1], sc[:, k, 1:2],
                            op0=mybir.AluOpType.is_equal,
                            op1=mybir.AluOpType.mult)
                        nc.tensor.matmul(pa[:, :], gt[:, k, :], seg[:],
                                         start=(mm == 0), stop=(mm == nmm - 1))
                        mm += 1
                live.append((tg, pa))
            for tg, pa in live:
                with nc.allow_low_precision(reason="praw is consumed by a "
                                            "bf16 matmul"):
                    nc.vector.tensor_copy(
                        praw[:, praw_off[tg] + w * P:
                             praw_off[tg] + (w + 1) * P], pa[:, :])
            pb = psB.tile([P, P], mybir.dt.float32, tag="out2")
            for ti, tg in enumerate(tags):
                R = rel_by_tag[tg]
                nc.tensor.matmul(
                    pb[:od, :cols], sb_W[l][:, R["r"], :],
                    praw[:, praw_off[tg] + w * P:praw_off[tg] + w * P + cols],
                    start=(ti == 0), stop=False)
            nc.tensor.matmul(pb[:od, :cols], sb_L[l][:], hT[l % 2][:, cs:ce],
                             start=False, stop=True)
            if l < 2:
                nc.scalar.activation(
                    hT[(l + 1) % 2][:od, cs:ce], pb[:od, :cols],
                    mybir.ActivationFunctionType.Relu, bias=sb_b[l][:])
                if dnt in SRC_NTYPES:
                    stage_ag_window(l + 1, dnt, w * P, cols)
            else:
                fin = wpool.tile([P, P], mybir.dt.float32, tag="fin")
                nc.scalar.activation(
                    fin[:od, :cols], pb[:od, :cols],
                    mybir.ActivationFunctionType.Identity, bias=sb_b[l][:])
                pt = pst.tile([P, P], mybir.dt.float32, tag="tp")
                nc.tensor.transpose(pt[:cols, :od], fin[:od, :cols],
                                    identity[:od, :od])
                stg = wpool.tile([P, DOUT], mybir.dt.float32, tag="ostg")
                nc.vector.tensor_copy(stg[:cols, :], pt[:cols, :od])
                nc.sync.dma_start(out_par[cs:ce, :], stg[:cols, :])

        def emit_phase(l, dnt, tags):
            for w in range(nw[dnt]):
                emit_window(l, dnt, tags, w)

        # ---- program ----
        # Layer schedule (l < 2): gene windows (each window gathers, reduces,
        # applies weights, and stages its next-layer AllGather input) ->
        # drug windows -> AllGather(gene, l+1) [gpsimd dispatch waits only on
        # gene windows, long since done; wire time overlaps the disease
        # gathers] -> disease windows -> AllGather(drug, l+1).
        emit_embedding("gene")
        emit_ag(0, "gene")
        emit_embedding("drug")
        emit_ag(0, "drug")
        emit_embedding("disease")
        for l in range(3):
            emit_phase(l, *PHASES[0])
            emit_phase(l, *PHASES[1])
            if l < 2:
                emit_ag_collective(l + 1, "gene")
            emit_phase(l, *PHASES[2])
            if l < 2:
                emit_ag_collective(l + 1, "drug")

    nc.compile()
    return nc


# ---------------------------------------------------------------------------
# entry point
# ---------------------------------------------------------------------------

def _install_ntff_hook():
    if "antenv.axon_hooks" in sys.modules:
        return
    mod = types.ModuleType("antenv.axon_hooks")
    mod._hook = None
    mod.set_axon_ntff_profile_hook = lambda h: setattr(mod, "_hook", h)
    mod.get_axon_ntff_profile_hook = lambda: mod._hook
    sys.modules["antenv.axon_hooks"] = mod
    try:
        import antenv
        antenv.axon_hooks = mod
        from trn_agent_boot.trn_boot import _ntff_profile_via_ctypes
        hook = _ntff_profile_via_ctypes("/opt/axon/libaxon_pjrt.so")
        if hook is not None:
            mod.set_axon_ntff_profile_hook(hook)
    except Exception:
        pass


def run(inputs, cfg=CFG, trace=False, tmpdir=None):
    S, percore = preprocess(cfg, inputs)
    nc = build(S)
    _install_ntff_hook()
    from concourse import bass_utils
    bass_utils.upload_artifacts = lambda d: d
    res = bass_utils.run_bass_kernel_spmd(
        nc, percore, list(range(cfg["NCORE"])), trace=trace, tmpdir=tmpdir,
        trace_cores=[0] if trace else None)
    ncore = cfg["NCORE"]
    shard = {nt: cfg["N"][nt] // ncore for nt in NTYPES}
    outs = []
    o = 0
    for nt in NTYPES:
        parts = [res.results[c]["out"][o:o + shard[nt]] for c in range(ncore)]
        outs.append(np.concatenate(parts, 0))
        o += shard[nt]
    full = np.concatenate(outs, 0).astype(np.float32)
    run.last_exec_time_ns = res.exec_time_ns
    return full


def kernel(**inputs):
    return run(inputs)


# revision 14
# speedup vs baseline: 3.3109x; 2.1545x over previous
"""Trainium2 Bass kernel for nn_BaseRGCNHetero (3-layer heterogeneous RGCN).

Strategy (8 NeuronCores, SPMD):
  - Destination-shard the nodes: core c owns rows [c*N/8, (c+1)*N/8) of every
    node type; all edges whose dst is in the shard are processed there, so
    per-relation aggregates need no cross-core reduction.
  - Aggregate-first algebra: agg[dst] = (sum_{e->dst} h[src]) @ W_r * inv_deg,
    sharing one bf16 DRAM gather table per source ntype (drug, gene) per
    layer.  After each layer the drug/gene h-shards are AllGathered (bf16)
    into the next layer's tables; each AllGather is issued as soon as its
    ntype's windows finish so the wire time overlaps the remaining gathers.
  - Edge slot stream per relation: 128-dst windows in natural order, one
    chunk per (window, src bank), slots sorted by src row and padded to a
    multiple of 128 (pad slots point at an all-zero table row).  Non-
    transposed dma_gather pulls h[src] rows node-major ([slot, feat] blocks);
    gathers round-robin over 4 SWDGE queues so descriptor generation runs on
    all four GpSimd Q7 core pairs concurrently (3.3x single-queue, and the
    non-transpose path avoids the xbar that makes concurrent transposed
    gathers corrupt each other).
  - Segment sums on TensorE: per 128-slot block, VectorE builds a one-hot
    seg matrix seg[slot, dst] = (dstcol[slot] == dst) * inv_deg[slot] from
    host-streamed per-block columns; matmul(gt_block^T @ seg) accumulates
    agg[feat, dst] for the window in PSUM across the window's blocks.
  - Per dst window: agg -> bf16 praw, then one matmul per relation (W_r) plus
    the self-loop h @ L accumulate in a single PSUM bank; bias (+relu) is a
    fused ScalarE activation per window.
"""
import sys
import types
import numpy as np
import ml_dtypes
from contextlib import ExitStack

import concourse.bass as bass
import concourse.bacc as bacc
import concourse.tile as tile
from concourse import mybir, library_config

BF16 = ml_dtypes.bfloat16
P = 128
NQ = 4             # SWDGE queues (gather descriptor-gen parallelism)

CFG = dict(
    N={"drug": 20000, "gene": 50000, "disease": 10000},
    MOD={"drug": 1024, "gene": 768, "disease": 512},
    D_IN=128, D_H=128, D_OUT=64,
    RELS=[("drug", "disease", "dd"), ("drug", "drug", "ddr"),
          ("drug", "gene", "dg"), ("gene", "disease", "gd"),
          ("gene", "gene", "gg")],
    NCORE=8,
    BANK=32768,     # dma_gather int16 row-index limit per table slice
)

NTYPES = ("drug", "gene", "disease")
SRC_NTYPES = ("drug", "gene")
# layer processing phases: dst ntype -> relations feeding it (tags)
PHASES = [("gene", ["dg", "gg"]), ("drug", ["ddr"]), ("disease", ["dd", "gd"])]


# ---------------------------------------------------------------------------
# host-side preprocessing
# ---------------------------------------------------------------------------

def _pack_idx(stream):
    """int array (len % 128 == 0) -> dma_gather idx layout [128, len/16] int16:
    idx i at (i%16, i//16), replicated across the 8 groups of 16 partitions."""
    n = stream.size
    v = stream.astype(np.int16).reshape(n // 16, 16).T
    return np.tile(v, (8, 1))


def _banks(cfg, snt):
    """Gather-table bank slices for source ntype snt.
    Table rows: 0 = zeros, 1..N = nodes, N+1 = zeros.
    Returns list of (start_row, end_row, pad_row_relative)."""
    n = cfg["N"][snt]
    trows = n + 2
    if trows <= cfg["BANK"]:
        return [(0, trows, 0)]
    return [(0, cfg["BANK"], 0), (cfg["BANK"], trows, n + 1 - cfg["BANK"])]


def preprocess(cfg, inputs):
    ncore = cfg["NCORE"]
    shard = {nt: cfg["N"][nt] // ncore for nt in NTYPES}
    nw = {nt: -(-shard[nt] // P) for nt in NTYPES}

    S = dict(cfg=cfg, nw=nw, shard=shard, rels=[])
    percore = [dict() for _ in range(ncore)]

    for r, (snt, dnt, tag) in enumerate(cfg["RELS"]):
        src = np.asarray(inputs["e_" + tag + "_s"]).astype(np.int64)
        dst = np.asarray(inputs["e_" + tag + "_d"]).astype(np.int64)
        banks = _banks(cfg, snt)
        nbank = len(banks)
        NW = nw[dnt]
        dsh = shard[dnt]

        core_of = dst // dsh
        deg_all = np.bincount(dst, minlength=cfg["N"][dnt]).astype(np.float32)
        inv_deg = 1.0 / np.maximum(deg_all, 1.0)

        row_all = src + 1
        bank_of = (row_all >= cfg["BANK"]).astype(np.int64) if nbank == 2 \
            else np.zeros(row_all.size, np.int64)

        # per-core per-(window, bank) edge counts -> shared block counts
        cnt = np.zeros((ncore, NW, nbank), np.int64)
        ld_all = dst - core_of * dsh
        w_all = ld_all // P
        for c in range(ncore):
            m = core_of == c
            key = w_all[m] * nbank + bank_of[m]
            cnt[c] = np.bincount(key, minlength=NW * nbank).reshape(NW, nbank)
        nblk = -(-cnt.max(axis=0) // P)          # [NW, nbank] shared
        slots_wb = nblk * P
        off_wb = np.zeros((NW, nbank), np.int64)
        gathers = []                              # (bank, off, slots, nblk, segoff)
        off = 0
        segoff = 0
        for w in range(NW):
            for b in range(nbank):
                if nblk[w, b] == 0:
                    continue
                off_wb[w, b] = off
                gathers.append((w, b, int(off), int(slots_wb[w, b]),
                                int(nblk[w, b]), int(segoff)))
                off += int(slots_wb[w, b])
                segoff += int(nblk[w, b])
        nslots = max(off, P)
        nblk_tot = max(segoff, 1)
        maxblk = int(nblk.max()) if nblk.size else 1

        for c in range(ncore):
            stream = np.zeros(nslots, np.int16)
            segm = np.zeros((nblk_tot, P, P), np.float32)
            for w, b, o, sl, nb, so in gathers:
                stream[o:o + sl] = banks[b][2]
            m = core_of == c
            e_row = row_all[m] - np.array([bk[0] for bk in banks])[bank_of[m]]
            e_b = bank_of[m]
            e_ld = ld_all[m]
            e_w = e_ld // P
            e_dl = e_ld % P
            e_iv = inv_deg[dst[m]]
            order = np.lexsort((e_row, e_b, e_w))
            key = (e_w * nbank + e_b)[order]
            starts = np.r_[0, np.flatnonzero(np.diff(key)) + 1]
            sizes = np.diff(np.r_[starts, key.size])
            rank = np.arange(key.size) - np.repeat(starts, sizes)
            pos = off_wb[e_w[order], e_b[order]] + rank
            stream[pos] = e_row[order].astype(np.int16)
            # seg[slot, dst] one-hot with inv_deg folded in, host-built so
            # the device never runs 16-bit DVE ops next to SWDGE (they lock
            # the descriptor rings and serialize the gathers)
            segm[pos // P, pos % P, e_dl[order]] = e_iv[order]
            percore[c][f"idx_{tag}"] = _pack_idx(stream)
            percore[c][f"seg_{tag}"] = np.ascontiguousarray(
                segm.transpose(1, 0, 2)).astype(BF16)

        S["rels"].append(dict(r=r, snt=snt, dnt=dnt, tag=tag, NW=NW,
                              banks=banks, gathers=gathers, nslots=nslots,
                              nblk_tot=nblk_tot, maxblk=maxblk))

    for nt in NTYPES:
        x = np.asarray(inputs["x_" + nt])
        for c in range(ncore):
            sh = shard[nt]
            percore[c][f"xT_{nt}"] = np.ascontiguousarray(
                x[c * sh:(c + 1) * sh].T).astype(BF16)

    com = dict()
    for nt in NTYPES:
        com[f"We_{nt}"] = np.asarray(inputs["We_" + nt]).astype(BF16)
        com[f"be_{nt}"] = np.asarray(inputs["be_" + nt]).astype(
            np.float32).reshape(-1, 1)
    for l in range(3):
        com[f"W{l}"] = np.asarray(inputs[f"W{l}"]).astype(BF16)
        com[f"L{l}"] = np.asarray(inputs[f"L{l}"]).astype(BF16)
        com[f"b{l}"] = np.asarray(inputs[f"b{l}"]).astype(np.float32).reshape(-1, 1)
    for c in range(ncore):
        percore[c].update(com)
    return S, percore


# ---------------------------------------------------------------------------
# device program
# ---------------------------------------------------------------------------

def build(S):
    cfg = S["cfg"]
    ncore = cfg["NCORE"]
    nw, shard = S["nw"], S["shard"]
    DH, DOUT = cfg["D_H"], cfg["D_OUT"]
    NREL = len(cfg["RELS"])
    nsh_tot = sum(shard.values())
    rel_by_tag = {R["tag"]: R for R in S["rels"]}
    maxblk_all = max(R["maxblk"] for R in S["rels"])
    # praw16 column offsets per phase (buffers reused across phases)
    praw_off = {}
    praw_cols = 0
    for dnt, tags in PHASES:
        o = 0
        for tg in tags:
            praw_off[tg] = o
            o += nw[dnt] * P
        praw_cols = max(praw_cols, o)

    nc = bacc.Bacc("TRN2", target_bir_lowering=False, debug=False,
                   num_devices=ncore, num_swdge_queues=NQ)

    par = {}
    for nt in NTYPES:
        par[f"xT_{nt}"] = nc.declare_dram_parameter(
            f"xT_{nt}", [cfg["MOD"][nt], shard[nt]], mybir.dt.bfloat16, False)
        par[f"We_{nt}"] = nc.declare_dram_parameter(
            f"We_{nt}", [cfg["MOD"][nt], cfg["D_IN"]], mybir.dt.bfloat16, False)
        par[f"be_{nt}"] = nc.declare_dram_parameter(
            f"be_{nt}", [cfg["D_IN"], 1], mybir.dt.float32, False)
    for l in range(3):
        od = DOUT if l == 2 else DH
        par[f"W{l}"] = nc.declare_dram_parameter(
            f"W{l}", [NREL, DH, od], mybir.dt.bfloat16, False)
        par[f"L{l}"] = nc.declare_dram_parameter(
            f"L{l}", [DH, od], mybir.dt.bfloat16, False)
        par[f"b{l}"] = nc.declare_dram_parameter(
            f"b{l}", [od, 1], mybir.dt.float32, False)
    for R in S["rels"]:
        tg = R["tag"]
        par[f"idx_{tg}"] = nc.declare_dram_parameter(
            f"idx_{tg}", [P, R["nslots"] // 16], mybir.dt.int16, False)
        par[f"seg_{tg}"] = nc.declare_dram_parameter(
            f"seg_{tg}", [P, R["nblk_tot"], P], mybir.dt.bfloat16, False)
    out_par = nc.declare_dram_parameter("out", [nsh_tot, DOUT],
                                        mybir.dt.float32, True)

    agin, tabs = {}, {}
    for l in range(3):
        for nt in SRC_NTYPES:
            agin[(l, nt)] = nc.dram_tensor(
                f"agin{l}_{nt}", [shard[nt], DH], mybir.dt.bfloat16)
            tabs[(l, nt)] = nc.dram_tensor(
                f"tab{l}_{nt}", [cfg["N"][nt] + 2, DH], mybir.dt.bfloat16,
                addr_space="Shared")

    with ExitStack() as ctx:
        tc = ctx.enter_context(tile.TileContext(nc))
        nc.gpsimd.load_library(library_config.mlp)

        const = ctx.enter_context(tc.tile_pool(name="const", bufs=1))
        persist = ctx.enter_context(tc.tile_pool(name="persist", bufs=1))
        gpool = ctx.enter_context(tc.tile_pool(name="gpool", bufs=6))
        ipool = ctx.enter_context(tc.tile_pool(name="ipool", bufs=8))
        xpool = ctx.enter_context(tc.tile_pool(name="xpool", bufs=2))
        wpool = ctx.enter_context(tc.tile_pool(name="wpool", bufs=4))
        spool = ctx.enter_context(tc.tile_pool(name="spool", bufs=6))
        pst = ctx.enter_context(tc.tile_pool(name="pst", bufs=2, space="PSUM"))
        psA = ctx.enter_context(tc.tile_pool(name="psA", bufs=2, space="PSUM"))
        psB = ctx.enter_context(tc.tile_pool(name="psB", bufs=2, space="PSUM"))
        psE = ctx.enter_context(tc.tile_pool(name="psE", bufs=2, space="PSUM"))

        identity = const.tile([P, P], mybir.dt.float32)
        from concourse.masks import make_identity
        make_identity(nc, identity[:])
        identity16 = const.tile([P, P], mybir.dt.bfloat16)
        nc.vector.tensor_copy(identity16[:], identity[:])

        sb_W, sb_L, sb_b = {}, {}, {}
        for l in range(3):
            od = DOUT if l == 2 else DH
            t = const.tile([DH, NREL, od], mybir.dt.bfloat16, tag=f"W{l}")
            nc.sync.dma_start(t[:], par[f"W{l}"][:].rearrange("r k o -> k r o"))
            sb_W[l] = t
            sb_L[l] = const.tile([DH, od], mybir.dt.bfloat16, tag=f"L{l}",
                                 name=f"L{l}")
            nc.sync.dma_start(sb_L[l][:], par[f"L{l}"][:])
            sb_b[l] = const.tile([od, 1], mybir.dt.float32, tag=f"b{l}",
                                 name=f"b{l}")
            nc.sync.dma_start(sb_b[l][:], par[f"b{l}"][:])

        zrow = const.tile([1, DH], mybir.dt.bfloat16)
        nc.vector.memset(zrow[:], 0.0)
        for l in range(3):
            for nt in SRC_NTYPES:
                n = cfg["N"][nt]
                nc.sync.dma_start(tabs[(l, nt)][0:1, :], zrow[:])
                nc.sync.dma_start(tabs[(l, nt)][n + 1:n + 2, :], zrow[:])

        hT = [persist.tile([DH, nsh_tot], mybir.dt.bfloat16, tag=f"hT{i}",
                           name=f"hT{i}")
              for i in range(2)]
        nt_off, o = {}, 0
        for nt in NTYPES:
            nt_off[nt] = o
            o += shard[nt]
        praw = persist.tile([DH, praw_cols], mybir.dt.bfloat16, tag="praw")

        gq_counter = [0]

        def emit_embedding(nt):
            mod, sh = cfg["MOD"][nt], shard[nt]
            kt = mod // P
            sb_we = xpool.tile([P, 8, cfg["D_IN"]], mybir.dt.bfloat16, tag="we")
            nc.sync.dma_start(
                sb_we[:, :kt, :],
                par[f"We_{nt}"][:].rearrange("(k p) f -> p k f", p=P))
            sb_be = wpool.tile([cfg["D_IN"], 1], mybir.dt.float32, tag="be")
            nc.sync.dma_start(sb_be[:], par[f"be_{nt}"][:])
            for n0 in range(0, sh, 512):
                n1 = min(n0 + 512, sh)
                cols = n1 - n0
                xt = xpool.tile([P, 8, 512], mybir.dt.bfloat16, tag="xt")
                nc.sync.dma_start(
                    xt[:, :kt, :cols],
                    par[f"xT_{nt}"][:].rearrange(
                        "(k p) n -> p k n", p=P)[:, :, n0:n1])
                pe = psE.tile([P, 512], mybir.dt.float32, tag="emb")
                for k in range(kt):
                    nc.tensor.matmul(pe[:, :cols], sb_we[:, k, :],
                                     xt[:, k, :cols],
                                     start=(k == 0), stop=(k == kt - 1))
                nc.scalar.activation(
                    hT[0][:, nt_off[nt] + n0:nt_off[nt] + n1], pe[:, :cols],
                    mybir.ActivationFunctionType.Identity, bias=sb_be[:])

        def stage_ag_window(l, nt, w0, cols):
            """Transpose one hT[l] window of ntype nt into the AllGather
            staging buffer."""
            src = hT[l % 2][:, nt_off[nt] + w0:nt_off[nt] + w0 + cols]
            pt = pst.tile([P, P], mybir.dt.bfloat16, tag="tp", name="pt16")
            nc.tensor.transpose(pt[:cols, :DH], src, identity16[:])
            stg = wpool.tile([P, DH], mybir.dt.bfloat16, tag="agstg")
            nc.vector.tensor_copy(stg[:cols, :], pt[:cols, :DH])
            nc.sync.dma_start(agin[(l, nt)][w0:w0 + cols, :], stg[:cols, :])

        def emit_ag_collective(l, nt):
            nc.gpsimd.collective_compute(
                "AllGather", mybir.AluOpType.bypass,
                replica_groups=[list(range(ncore))],
                ins=[agin[(l, nt)][:]],
                outs=[tabs[(l, nt)][1:cfg["N"][nt] + 1]],
            )

        def emit_ag(l, nt):
            for w0 in range(0, shard[nt], P):
                stage_ag_window(l, nt, w0, min(P, shard[nt] - w0))
            emit_ag_collective(l, nt)

        def emit_window(l, dnt, tags, w):
            """One dst window: per-relation gathers + seg matmuls into PSUM
            agg -> praw, then W_r matmuls + self-loop + activation."""
            od = DOUT if l == 2 else DH
            sh = shard[dnt]
            cs = nt_off[dnt] + w * P
            ce = min(cs + P, nt_off[dnt] + sh)
            cols = ce - cs
            live = []
            for tg in tags:
                R = rel_by_tag[tg]
                gs = [g for g in R["gathers"] if g[0] == w]
                nmm = sum(g[4] for g in gs)
                if nmm == 0:
                    nc.vector.memset(
                        praw[:, praw_off[tg] + w * P:
                         praw_off[tg] + (w + 1) * P], 0.0)
                    continue
                pa = psA.tile([P, P], mybir.dt.float32, tag="agg")
                mm = 0
                for (_, b, soff, slots, nb, segoff) in gs:
                    sbi = ipool.tile([P, maxblk_all * P // 16],
                                     mybir.dt.int16, tag="idx")
                    nc.sync.dma_start(
                        sbi[:, :slots // 16],
                        par[f"idx_{tg}"][:, soff // 16:(soff + slots) // 16])
                    gt = gpool.tile([P, maxblk_all, P], mybir.dt.bfloat16,
                                    tag="gat")
                    b0, b1, _ = R["banks"][b]
                    q = gq_counter[0] % NQ
                    gq_counter[0] += 1
                    nc.gpsimd.dma_gather(
                        out_ap=gt[:, :nb, :], in_ap=tabs[(l, R["snt"])][b0:b1],
                        idxs_ap=sbi[:, :slots // 16],
                        num_idxs=slots, num_idxs_reg=slots,
                        elem_size=DH, transpose=False, single_packet=False,
                        queue_num=q)
                    sg = spool.tile([P, maxblk_all, P], mybir.dt.bfloat16,
                                    tag="sg")
                    nc.sync.dma_start(
                        sg[:, :nb, :],
                        par[f"seg_{tg}"][:, segoff:segoff + nb, :])
                    for k in range(nb):
                        nc.tensor.matmul(pa[:, :], gt[:, k, :], sg[:, k, :],
                                         start=(mm == 0), stop=(mm == nmm - 1))
                        mm += 1
                live.append((tg, pa))
            for tg, pa in live:
                with nc.allow_low_precision(reason="praw is consumed by a "
                                            "bf16 matmul"):
                    nc.vector.tensor_copy(
                        praw[:, praw_off[tg] + w * P:
                             praw_off[tg] + (w + 1) * P], pa[:, :])
            pb = psB.tile([P, P], mybir.dt.float32, tag="out2")
            for ti, tg in enumerate(tags):
                R = rel_by_tag[tg]
                nc.tensor.matmul(
                    pb[:od, :cols], sb_W[l][:, R["r"], :],
                    praw[:, praw_off[tg] + w * P:praw_off[tg] + w * P + cols],
                    start=(ti == 0), stop=False)
            nc.tensor.matmul(pb[:od, :cols], sb_L[l][:], hT[l % 2][:, cs:ce],
                             start=False, stop=True)
            if l < 2:
                nc.scalar.activation(
                    hT[(l + 1) % 2][:od, cs:ce], pb[:od, :cols],
                    mybir.ActivationFunctionType.Relu, bias=sb_b[l][:])
                if dnt in SRC_NTYPES:
                    stage_ag_window(l + 1, dnt, w * P, cols)
            else:
                fin = wpool.tile([P, P], mybir.dt.float32, tag="fin")
                nc.scalar.activation(
                    fin[:od, :cols], pb[:od, :cols],
                    mybir.ActivationFunctionType.Identity, bias=sb_b[l][:])
                pt = pst.tile([P, P], mybir.dt.float32, tag="tp")
                nc.tensor.transpose(pt[:cols, :od], fin[:od, :cols],
                                    identity[:od, :od])
                stg = wpool.tile([P, DOUT], mybir.dt.float32, tag="ostg")
                nc.vector.tensor_copy(stg[:cols, :], pt[:cols, :od])
                nc.sync.dma_start(out_par[cs:ce, :], stg[:cols, :])

        def emit_phase(l, dnt, tags):
            for w in range(nw[dnt]):
                emit_window(l, dnt, tags, w)

        # ---- program ----
        # Layer schedule (l < 2): gene windows (each window gathers, reduces,
        # applies weights, and stages its next-layer AllGather input) ->
        # drug windows -> AllGather(gene, l+1) [gpsimd dispatch waits only on
        # gene windows, long since done; wire time overlaps the disease
        # gathers] -> disease windows -> AllGather(drug, l+1).
        emit_embedding("gene")
        emit_ag(0, "gene")
        emit_embedding("drug")
        emit_ag(0, "drug")
        emit_embedding("disease")
        for l in range(3):
            emit_phase(l, *PHASES[0])
            emit_phase(l, *PHASES[1])
            if l < 2:
                emit_ag_collective(l + 1, "gene")
            emit_phase(l, *PHASES[2])
            if l < 2:
                emit_ag_collective(l + 1, "drug")

    nc.compile()
    return nc


# ---------------------------------------------------------------------------
# entry point
# ---------------------------------------------------------------------------

def _install_ntff_hook():
    if "antenv.axon_hooks" in sys.modules:
        return
    mod = types.ModuleType("antenv.axon_hooks")
    mod._hook = None
    mod.set_axon_ntff_profile_hook = lambda h: setattr(mod, "_hook", h)
    mod.get_axon_ntff_profile_hook = lambda: mod._hook
    sys.modules["antenv.axon_hooks"] = mod
    try:
        import antenv
        antenv.axon_hooks = mod
        from trn_agent_boot.trn_boot import _ntff_profile_via_ctypes
        hook = _ntff_profile_via_ctypes("/opt/axon/libaxon_pjrt.so")
        if hook is not None:
            mod.set_axon_ntff_profile_hook(hook)
    except Exception:
        pass


def run(inputs, cfg=CFG, trace=False, tmpdir=None):
    S, percore = preprocess(cfg, inputs)
    nc = build(S)
    _install_ntff_hook()
    from concourse import bass_utils
    bass_utils.upload_artifacts = lambda d: d
    res = bass_utils.run_bass_kernel_spmd(
        nc, percore, list(range(cfg["NCORE"])), trace=trace, tmpdir=tmpdir,
        trace_cores=[0] if trace else None)
    ncore = cfg["NCORE"]
    shard = {nt: cfg["N"][nt] // ncore for nt in NTYPES}
    outs = []
    o = 0
    for nt in NTYPES:
        parts = [res.results[c]["out"][o:o + shard[nt]] for c in range(ncore)]
        outs.append(np.concatenate(parts, 0))
        o += shard[nt]
    full = np.concatenate(outs, 0).astype(np.float32)
    run.last_exec_time_ns = res.exec_time_ns
    return full


def kernel(**inputs):
    return run(inputs)
